# revision 61
# baseline (speedup 1.0000x reference)
"""Trainium2 Bass kernel for nn_Amplified_PatternMixer.

Computation:
  out[b, h, m1, m2] = mixed_pattern[h, m1, m2] + alpha[h] * nrm[b, m2]
where
  nrm[b, m] = || mean_{hw}(x[b*57+m, :, h, w]) ||_2   over channels
  mixed_pattern = tiny 57x57 graph-normalized pattern (from 5x7x7 params).

The memory-bound part (streaming x: [1824, 256, 14, 14]) runs on 8
NeuronCores, data-parallel over rows (228 rows/core).

Optimization history / HW facts (measured on this trn2):
  * f32 stream was DMA-bound at ~130-156us (45.8 MB/core, ~23-27 B/ns
    per SDMA engine x16).
  * Upload dtype is ours to choose: host casts x to bf16 (RNE).  The
    pooled-mean + channel-norm averages away the 0.4% per-element
    quantization noise (measured end-to-end rel err ~2e-4 vs the 2e-2
    gate).  Halves DMA bytes -> ~62us floor.
  * DVE TENSOR_REDUCE only has a 1x uop (1 elem/cycle/lane at ANY
    dtype; bf16 run measured 1.064 ns/elem, same as f32), so a plain
    reduce is a 93us floor -> restructure as a TT tree: bf16
    tensor_tensor DOES hit the 2x_1P packed mode (2 outs = 4 ins per
    cycle, measured 0.55 cyc/out), but only while every src/dst is
    2-byte, step +-1 and 4-byte aligned.  196 = 4*49 breaks alignment
    at the second level, so the host pads each channel 196 -> 200
    zeros (+2% DMA): L1 200->100 and L2 100->50 both stay packed, the
    final 50-wide reduce runs at 1x.  Measured 4626 ns per 32-channel
    tile = 145 ns/channel-column -> ~66us DVE for the full core load.
  * GpSimd tensor_add (bf16) measured 175 ns/channel-column solo, but
    POOL shares an SBUF port with DVE: any POOL activity overlapping a
    packed DVE TT inflates the TT 2-4x (and POOL itself slows to ~290
    ns/ch), so offloading L1 folds to POOL is NET NEGATIVE; the final
    kernel keeps pool_share=0 (the code path remains, gated by
    K_PC_NUM, with a gt(t-1)-read dummy that phase-locks POOL to the
    contention-immune reduce window -- measured still not worth it).
  * ScalarE ACTIVATE+READ_ACCUMULATOR costs ~740 ns per accumulate
    instruction pair regardless of size -- 5x worse than DVE per
    channel, BUT ScalarE has its OWN SBUF port (no DVE contention,
    unlike POOL), so besides the per-tile Square(cs) partials it also
    sums the last 10 (w=64) / 5 (w=32) channels of every big tile raw
    (one Copy+accum per channel, accumulating bf16 results into spare
    columns of the same cs tile the DVE reduce fills, so ONE Square
    covers the whole tile); ~60 channel-columns offloaded brings
    ScalarE to ~56us busy, co-critical with DVE at ~58us -- worth
    ~7us off the span (85 -> 78us quiet).
  * All input DMA doorbells go on the sync ring: with rings
    alternating sync/scalar, the scalar-queued doorbells serialize
    behind the Square activations (which wait on DVE reduces),
    coupling DMA issue to DVE progress -- sync-only measured both
    faster and far less variance.  Exception: the first four ramp
    tiles alternate onto the scalar ring (no Squares are queued there
    yet, so no coupling) to land the ramp earlier.  Output DMAs live
    on the scalar ring (they naturally follow the Squares they depend
    on), split into a mid-stream flush + a small final tail.
  * fp8 would halve DMA again but no engine reduces fp8 faster than
    1 elem/cycle (DVE fp8 TT measured 1.04 cyc/out, no packing; PE
    LDW+MM measured 104+165 ns per 128x128 block = way under the
    needed rate), so fp8 upload is compute-dead.  bf16 is optimal.
  * Deeper pad-208 tree (L1/L2/L3/R26) measures the same as pad-200
    (extra instruction overhead cancels the reduce saving); kept
    selectable via K_WP.
  * Steady state: DVE ~58us busy and ScalarE ~56us busy co-critical
    over a ~57us DMA stream; span adds ~7us fixed framework prologue
    + ~3us first-tile flight + ~2us epilogue.  Best measured ~78us in
    quiet HBM windows; shared-HBM neighbor bursts add +10us.
"""

import ml_dtypes
import numpy as np

import concourse.bacc as bacc
import concourse.mybir as mybir
import concourse.tile as tile
from concourse.bass_utils import run_bass_kernel_spmd

# Problem constants (hardcoded; kernel.py must be self-contained).
NUM_BASIC = 5
NUM_MIXED = 4
NUM_FRAME = 8
NUM_NODES = 7
NUM_SAMPLES = 8
M = 1 + NUM_NODES * NUM_FRAME  # 57

N_CORES = 8
B = 32
C = 256
HW = 196   # 14*14 (true)
import os as _os
WP = int(_os.environ.get("K_WP", "200"))
# host-padded channel width (zeros) keeping the TT tree 4B-aligned at
# every level: 208 -> 104 -> 52 -> 26 (reduce 26-wide), or the
# shallower 200 -> 100 -> 50 (reduce 50-wide).
assert WP in (200, 208)
ROWS_TOTAL = B * M          # 1824
ROWS_PER_CORE = ROWS_TOTAL // N_CORES  # 228
CWP = C * WP                # 51200 bf16 per row (padded)

# (row_start, w, pool_ch): tile covers rows row_start..row_start+w//2 as
# 128 pieces of w channels (w/2 rows x 256/w pieces); per-partition DMA
# run = w*200*2 B, one descriptor per partition.  pool_ch leading
# channels of each tile are L1-folded on GpSimd instead of DVE.
# Ramp lets DVE start early; 64-wide body maximizes descriptor size
# (25.6 KB) for DMA efficiency; taper keeps the drain short.
_WIDTHS_STR = _os.environ.get(
    "K_WIDTHS", "4,8,16,16,32,32,64,64,64,64,32,32,16,8,4"
)
_WIDTHS = [int(v) for v in _WIDTHS_STR.split(",")]
_BUFS = int(_os.environ.get("K_BUFS", "6"))
_PC_NUM = int(_os.environ.get("K_PC_NUM", "0"))  # pool share = _PC_NUM*w//16
_RINGS = tuple(_os.environ.get("K_RINGS", "sync").split(","))
assert sum(_WIDTHS) == 2 * ROWS_PER_CORE
assert all(C % w == 0 for w in _WIDTHS)


def _pool_share(w):
    # GpSimd folds a slice of big-tile channels.  POOL's tensor_add
    # contends with DVE's 2x-packed TENSOR_TENSOR for the shared SBUF
    # port (measured 2-4x TT slowdown when overlapped) but leaves the
    # 1x TENSOR_REDUCE untouched, so the share is sized to fit inside
    # the previous tile's reduce window even at POOL's contended rate
    # (~290 ns/channel measured): 12 channels ~ 3.5us for w=64.
    return (_PC_NUM * w // 16) if w >= 32 else 0


TILE_PLAN = []
_r = 0
for _w in _WIDTHS:
    TILE_PLAN.append((_r, _w, _pool_share(_w)))
    _r += _w // 2
N_TILES = len(TILE_PLAN)

LAST_RESULT = None
_NC_CACHE = None


def _build_nc(plan=TILE_PLAN, bufs=_BUFS, rings=_RINGS):
    # enable_partition_id=False: the SPMD program is identical on every
    # core and never reads partition_id, and skipping it removes a
    # runtime parameter-injection DMA that the start barrier waits on.
    nc = bacc.Bacc(None, enable_partition_id=False)
    x = nc.declare_dram_parameter(
        "x", [ROWS_PER_CORE, CWP], mybir.dt.bfloat16, isOutput=False
    )
    out = nc.declare_dram_parameter(
        "out", [128, len(plan)], mybir.dt.float32, isOutput=True
    )
    max_w = max(w for _, w, _ in plan)
    with tile.TileContext(nc) as tc:
        with (
            tc.tile_pool(name="xt_pool", bufs=bufs) as xp,
            tc.tile_pool(name="ft_pool", bufs=3) as fp,
            tc.tile_pool(name="gt_pool", bufs=2) as gp,
            tc.tile_pool(name="acc_pool", bufs=2) as accp,
            tc.tile_pool(name="res_pool", bufs=1) as resp,
        ):
            osb = resp.tile([128, len(plan)], mybir.dt.float32, tag="osb")

            h1, h2, h3 = WP // 2, WP // 4, WP // 8

            def finish(t, w, wd, g3, cs):
                # Final narrow reduce (1x) into cs[:, :wd]; the ACT-
                # offloaded channels already accumulated into
                # cs[:, wd:w], so ONE Square covers the whole tile.
                with nc.allow_low_precision("bf16 sums; fp32 internal accum"):
                    nc.vector.reduce_sum(cs[:, :wd], g3, axis=mybir.AxisListType.X)
                tr = accp.tile([128, max_w], mybir.dt.float32, tag="tr")
                nc.scalar.activation(
                    tr[:, :w],
                    cs[:, :w],
                    mybir.ActivationFunctionType.Square,
                    accum_out=osb[:, t : t + 1],
                )

            # Software-pipelined by one tile, with the DVE queue ordered
            # [reduce(t-1), L1b(t), L2(t)]: GpSimd's L1a(t) fold starts
            # when DMA(t) lands, which is when DVE starts the (port-
            # light, contention-immune) reduce of t-1 -- so POOL never
            # overlaps a packed TT, and DVE never stalls on the POOL
            # semaphore (L2(t) issues after both L1 halves).
            pdum = resp.tile([1, 4], mybir.dt.bfloat16, tag="pdum")
            T = len(plan)
            pending = None
            for t, (r0, w, pc) in enumerate(plan):
                b = C // w          # pieces per row
                f = w * WP          # bf16 elems per partition
                # The scalar engine (own SBUF port -- no DVE contention,
                # unlike POOL) sums the last w/8 channels of big tiles
                # raw via Copy+accum (739 ns/channel measured), shaving
                # the DVE critical path.
                ac = (10 if w == 64 else 5) if w >= 32 else 0
                xt = xp.tile([128, max_w * WP], mybir.dt.bfloat16, tag="xt")
                # The scalar ring is only coupled to DVE progress once
                # Squares enter its queue (first Square issues after
                # R(t0)), so the first few ramp tiles can use it as a
                # second ring to land earlier; everything after goes on
                # sync only.
                if t < 4:
                    eng = getattr(nc, ("sync", "scalar")[t % 2])
                else:
                    eng = getattr(nc, rings[t % len(rings)])
                src = x[r0 : r0 + w // 2, :].rearrange("a (b f) -> (a b) f", b=b)
                eng.dma_start(out=xt[:, :f], in_=src)

                x3 = xt[:, :f].rearrange("p (g v) -> p g v", v=WP)
                wd = w - ac         # channels on the DVE tree
                cs = accp.tile([128, max_w], mybir.dt.bfloat16, tag="cs")
                if ac:
                    trc = accp.tile([128, WP], mybir.dt.float32, tag="trc")
                    with nc.allow_low_precision("bf16 sums; fp32 internal accum"):
                        for j in range(ac):
                            nc.scalar.activation(
                                trc[:, :WP],
                                x3[:, wd + j, :],
                                mybir.ActivationFunctionType.Copy,
                                accum_out=cs[:, wd + j : wd + j + 1],
                            )
                ft = fp.tile([128, max_w * h1], mybir.dt.bfloat16, tag="ft")
                f3 = ft[:, : wd * h1].rearrange("p (g v) -> p g v", v=h1)
                # L1a: GpSimd folds the first pc channels.  The dummy
                # read of gt(t-1) gates the fold on L2(t-1) completion,
                # phase-locking POOL to DVE's (contention-immune) reduce
                # window -- an unconstrained POOL start drifts onto DVE's
                # packed TTs and halves their throughput (shared SBUF
                # port, measured 2-4x TT inflation).
                if pc:
                    if pending is not None:
                        gprev = pending[5]
                        nc.gpsimd.tensor_add(
                            pdum[0:1, 0:4], gprev[0:1, 0:4], gprev[0:1, 0:4]
                        )
                    nc.gpsimd.tensor_add(
                        f3[:, 0:pc, :], x3[:, 0:pc, 0:h1], x3[:, 0:pc, h1:WP]
                    )
                if pending is not None:
                    finish(*pending[:5])
                # L1b: DVE folds the rest (2x packed TT).
                if pc < wd:
                    nc.vector.tensor_add(
                        f3[:, pc:wd, :], x3[:, pc:wd, 0:h1], x3[:, pc:wd, h1:WP]
                    )
                # L2: h1 -> h2 (DVE, packed; waits on both L1 halves).
                gt = gp.tile([128, max_w * h2], mybir.dt.bfloat16, tag="gt")
                g3 = gt[:, : wd * h2].rearrange("p (g v) -> p g v", v=h2)
                nc.vector.tensor_add(g3, f3[:, :, 0:h2], f3[:, :, h2:h1])
                pending = (t, w, wd, g3, cs, gt)
                # Flush the early osb columns mid-stream so the final
                # output DMA only covers the last few tiles.  Issued on
                # the SCALAR ring: there it sits right after the Squares
                # it depends on; on the sync ring it would queue the
                # remaining input doorbells behind compute.
                if t == len(plan) - 3:
                    nsplit = len(plan) - 4
                    nc.scalar.dma_start(
                        out=out[0:128, 0:nsplit], in_=osb[:, 0:nsplit]
                    )
            finish(*pending[:5])
            nsplit = len(plan) - 4
            nc.scalar.dma_start(
                out=out[0:128, nsplit:], in_=osb[:, nsplit:T]
            )
    nc.finalize()
    return nc


def _get_nc():
    global _NC_CACHE
    if _NC_CACHE is None:
        _NC_CACHE = _build_nc()
    return _NC_CACHE


def _norms_from_partials(partials):
    """partials: [128, N_TILES] per-core -> per-row norms [228].

    Each tile's column is the full per-partition sum of squares: the
    DVE-reduced channels and the ACT-offloaded channels both land in
    the same cs tile before the single per-tile Square.
    """
    nsq = np.zeros(ROWS_PER_CORE, dtype=np.float64)
    for t, (r0, w, _) in enumerate(TILE_PLAN):
        b = C // w
        ps = partials[:, t].astype(np.float64).reshape(w // 2, b).sum(axis=1)
        nsq[r0 : r0 + w // 2] += ps
    return np.sqrt(nsq) / float(HW)


def _zero_mask():
    mask = np.ones((M, M), dtype=np.float64)
    for i in range(NUM_SAMPLES):
        r = (1 + i) * NUM_NODES
        for c in range(1, M):
            if c % NUM_NODES != 0 and (c - 1) // NUM_NODES != i:
                mask[r, c] = 0.0
    return mask


def _pattern_mixer_np(mat, sigma, lin_w, lin_b, mixed_mat):
    mat = np.asarray(mat, np.float64)            # [5, 7, 7]
    sigma = np.asarray(sigma, np.float64)        # [4, 5, 1]
    lin_w = np.asarray(lin_w, np.float64)        # [4, 5]
    lin_b = np.asarray(lin_b, np.float64)        # [4]
    mixed_mat = np.asarray(mixed_mat, np.float64)  # [4, 57, 57]

    T2 = 2 * NUM_FRAME - 1  # 15
    dist = np.abs(np.arange(T2, dtype=np.float64) - (NUM_FRAME - 1))
    te = (1.0 / (np.sqrt(2.0 * np.pi) * sigma)) * np.exp(
        -(dist**2) / (2.0 * sigma**2)
    )  # [4, 5, 15]
    ce = 1.0 / (1.0 + np.exp(-te))
    mixed = (
        np.einsum("hbt,bnm,hb->hntm", ce, mat, lin_w)
        + lin_b[:, None, None, None]
    )
    mixed = np.maximum(mixed, 0.0).reshape(NUM_MIXED, NUM_NODES, T2 * NUM_NODES)
    blocks = [
        mixed[
            :,
            :,
            NUM_NODES * (NUM_SAMPLES - 1 - i) : NUM_NODES * (2 * NUM_SAMPLES - 1 - i),
        ]
        for i in range(NUM_SAMPLES)
    ]
    add_block = np.concatenate(blocks, axis=1)  # [4, 56, 56]
    mm = mixed_mat.copy()
    mm[:, 1:, 1:] += add_block
    mm *= _zero_mask()[None]
    deg = np.maximum(mm.sum(axis=2), 1.0) ** -0.5  # [4, 57]
    return (deg[:, :, None] * mm * deg[:, None, :]).astype(np.float32)


def kernel(mat, x, sigma, lin_w, lin_b, mixed_mat, alpha):
    global LAST_RESULT
    xf = np.asarray(x, dtype=np.float32).reshape(ROWS_TOTAL, C, HW)
    xs = np.zeros((ROWS_TOTAL, C, WP), dtype=ml_dtypes.bfloat16)
    xs[:, :, :HW] = xf.astype(ml_dtypes.bfloat16)
    xs = xs.reshape(ROWS_TOTAL, CWP)
    in_maps = [
        {"x": xs[i * ROWS_PER_CORE : (i + 1) * ROWS_PER_CORE]} for i in range(N_CORES)
    ]
    nc = _get_nc()
    res = run_bass_kernel_spmd(nc, in_maps, core_ids=list(range(N_CORES)))
    LAST_RESULT = res
    norms = np.concatenate([_norms_from_partials(r["out"]) for r in res.results])
    nrm = norms.reshape(B, M).astype(np.float32)

    mp = _pattern_mixer_np(mat, sigma, lin_w, lin_b, mixed_mat)  # [4, 57, 57] f32
    alpha = np.asarray(alpha, np.float32).reshape(1, NUM_MIXED, 1, 1)
    out = mp[None] + alpha * nrm[:, None, None, :]
    return np.ascontiguousarray(out.astype(np.float32))


# revision 62
# speedup vs baseline: 1.0260x; 1.0260x over previous
"""Trainium2 Bass kernel for nn_Amplified_PatternMixer.

Computation:
  out[b, h, m1, m2] = mixed_pattern[h, m1, m2] + alpha[h] * nrm[b, m2]
where
  nrm[b, m] = || mean_{hw}(x[b*57+m, :, h, w]) ||_2   over channels
  mixed_pattern = tiny 57x57 graph-normalized pattern (from 5x7x7 params).

The memory-bound part (streaming x: [1824, 256, 14, 14]) runs on 8
NeuronCores, data-parallel over rows (228 rows/core).

Optimization history / HW facts (measured on this trn2):
  * f32 stream was DMA-bound at ~130-156us (45.8 MB/core, ~23-27 B/ns
    per SDMA engine x16).
  * Upload dtype is ours to choose: host casts x to bf16 (RNE).  The
    pooled-mean + channel-norm averages away the 0.4% per-element
    quantization noise (measured end-to-end rel err ~2e-4 vs the 2e-2
    gate).  Halves DMA bytes -> ~62us floor.
  * DVE TENSOR_REDUCE only has a 1x uop (1 elem/cycle/lane at ANY
    dtype; bf16 run measured 1.064 ns/elem, same as f32), so a plain
    reduce is a 93us floor -> restructure as a TT tree: bf16
    tensor_tensor DOES hit the 2x_1P packed mode (2 outs = 4 ins per
    cycle, measured 0.55 cyc/out), but only while every src/dst is
    2-byte, step +-1 and 4-byte aligned.  196 = 4*49 breaks alignment
    at the second level, so the host pads each channel 196 -> 200
    zeros (+2% DMA): L1 200->100 and L2 100->50 both stay packed, the
    final 50-wide reduce runs at 1x.  Measured 4626 ns per 32-channel
    tile = 145 ns/channel-column -> ~66us DVE for the full core load.
  * GpSimd tensor_add (bf16) measured 175 ns/channel-column solo, but
    POOL shares an SBUF port with DVE: any POOL activity overlapping a
    packed DVE TT inflates the TT 2-4x (and POOL itself slows to ~290
    ns/ch), so offloading L1 folds to POOL is NET NEGATIVE; the final
    kernel keeps pool_share=0 (the code path remains, gated by
    K_PC_NUM, with a gt(t-1)-read dummy that phase-locks POOL to the
    contention-immune reduce window -- measured still not worth it).
  * ScalarE ACTIVATE+READ_ACCUMULATOR costs ~740 ns per accumulate
    instruction pair regardless of size -- 5x worse than DVE per
    channel, BUT ScalarE has its OWN SBUF port (no DVE contention,
    unlike POOL), so besides the per-tile Square(cs) partials it also
    sums the last 10 (w=64) / 5 (w=32) channels of every big tile raw
    (one Copy+accum per channel, accumulating bf16 results into spare
    columns of the same cs tile the DVE reduce fills, so ONE Square
    covers the whole tile); ~60 channel-columns offloaded brings
    ScalarE to ~56us busy, co-critical with DVE at ~58us -- worth
    ~7us off the span (85 -> 78us quiet).
  * All input DMA doorbells go on the sync ring: with rings
    alternating sync/scalar, the scalar-queued doorbells serialize
    behind the Square activations (which wait on DVE reduces),
    coupling DMA issue to DVE progress -- sync-only measured both
    faster and far less variance.  Exception: the first four ramp
    tiles alternate onto the scalar ring (no Squares are queued there
    yet, so no coupling) to land the ramp earlier.  Output DMAs live
    on the scalar ring (they naturally follow the Squares they depend
    on), split into a mid-stream flush + a small final tail.
  * fp8 would halve DMA again but no engine reduces fp8 faster than
    1 elem/cycle (DVE fp8 TT measured 1.04 cyc/out, no packing; PE
    LDW+MM measured 104+165 ns per 128x128 block = way under the
    needed rate), so fp8 upload is compute-dead.  bf16 is optimal.
  * Deeper pad-208 tree (L1/L2/L3/R26) measures the same as pad-200
    (extra instruction overhead cancels the reduce saving); kept
    selectable via K_WP.
  * Steady state: DVE ~58us busy and ScalarE ~56us busy co-critical
    over a ~57us DMA stream; span adds ~7us fixed framework prologue
    + ~3us first-tile flight + ~2us epilogue.  Best measured ~78us in
    quiet HBM windows; shared-HBM neighbor bursts add +10us.
"""

import ml_dtypes
import numpy as np

import concourse.bacc as bacc
import concourse.mybir as mybir
import concourse.tile as tile
from concourse.bass_utils import run_bass_kernel_spmd

# Problem constants (hardcoded; kernel.py must be self-contained).
NUM_BASIC = 5
NUM_MIXED = 4
NUM_FRAME = 8
NUM_NODES = 7
NUM_SAMPLES = 8
M = 1 + NUM_NODES * NUM_FRAME  # 57

N_CORES = 8
B = 32
C = 256
HW = 196   # 14*14 (true)
import os as _os
WP = int(_os.environ.get("K_WP", "200"))
# host-padded channel width (zeros) keeping the TT tree 4B-aligned at
# every level: 208 -> 104 -> 52 -> 26 (reduce 26-wide), or the
# shallower 200 -> 100 -> 50 (reduce 50-wide).
assert WP in (200, 208)
ROWS_TOTAL = B * M          # 1824
ROWS_PER_CORE = ROWS_TOTAL // N_CORES  # 228
CWP = C * WP                # 51200 bf16 per row (padded)

# (row_start, w, pool_ch): tile covers rows row_start..row_start+w//2 as
# 128 pieces of w channels (w/2 rows x 256/w pieces); per-partition DMA
# run = w*200*2 B, one descriptor per partition.  pool_ch leading
# channels of each tile are L1-folded on GpSimd instead of DVE.
# Ramp lets DVE start early; 64-wide body maximizes descriptor size
# (25.6 KB) for DMA efficiency; taper keeps the drain short.
_WIDTHS_STR = _os.environ.get(
    "K_WIDTHS", "4,8,16,16,32,32,64,64,64,64,32,32,16,8,4"
)
_WIDTHS = [int(v) for v in _WIDTHS_STR.split(",")]
_BUFS = int(_os.environ.get("K_BUFS", "6"))
_PC_NUM = int(_os.environ.get("K_PC_NUM", "0"))  # pool share = _PC_NUM*w//16
_RINGS = tuple(_os.environ.get("K_RINGS", "sync").split(","))
assert sum(_WIDTHS) == 2 * ROWS_PER_CORE
assert all(C % w == 0 for w in _WIDTHS)


def _pool_share(w):
    # GpSimd folds a slice of big-tile channels.  POOL's tensor_add
    # contends with DVE's 2x-packed TENSOR_TENSOR for the shared SBUF
    # port (measured 2-4x TT slowdown when overlapped) but leaves the
    # 1x TENSOR_REDUCE untouched, so the share is sized to fit inside
    # the previous tile's reduce window even at POOL's contended rate
    # (~290 ns/channel measured): 12 channels ~ 3.5us for w=64.
    return (_PC_NUM * w // 16) if w >= 32 else 0


TILE_PLAN = []
_r = 0
for _w in _WIDTHS:
    TILE_PLAN.append((_r, _w, _pool_share(_w)))
    _r += _w // 2
N_TILES = len(TILE_PLAN)

LAST_RESULT = None
_NC_CACHE = None


def _build_nc(plan=TILE_PLAN, bufs=_BUFS, rings=_RINGS):
    # enable_partition_id=False: the SPMD program is identical on every
    # core and never reads partition_id, and skipping it removes a
    # runtime parameter-injection DMA that the start barrier waits on.
    nc = bacc.Bacc(None, enable_partition_id=False)
    x = nc.declare_dram_parameter(
        "x", [ROWS_PER_CORE, CWP], mybir.dt.bfloat16, isOutput=False
    )
    out = nc.declare_dram_parameter(
        "out", [128, len(plan)], mybir.dt.float32, isOutput=True
    )
    max_w = max(w for _, w, _ in plan)
    with tile.TileContext(nc) as tc:
        with (
            tc.tile_pool(name="xt_pool", bufs=bufs) as xp,
            tc.tile_pool(name="ft_pool", bufs=int(_os.environ.get("K_FT", "3"))) as fp,
            tc.tile_pool(name="gt_pool", bufs=2) as gp,
            tc.tile_pool(name="acc_pool", bufs=2) as accp,
            tc.tile_pool(name="res_pool", bufs=1) as resp,
        ):
            osb = resp.tile([128, len(plan)], mybir.dt.float32, tag="osb")

            h1, h2, h3 = WP // 2, WP // 4, WP // 8

            def finish(t, w, wd, g3, cs):
                # Final narrow reduce (1x) into cs[:, :wd]; the ACT-
                # offloaded channels already accumulated into
                # cs[:, wd:w], so ONE Square covers the whole tile.
                with nc.allow_low_precision("bf16 sums; fp32 internal accum"):
                    nc.vector.reduce_sum(cs[:, :wd], g3, axis=mybir.AxisListType.X)
                tr = accp.tile([128, max_w], mybir.dt.float32, tag="tr")
                nc.scalar.activation(
                    tr[:, :w],
                    cs[:, :w],
                    mybir.ActivationFunctionType.Square,
                    accum_out=osb[:, t : t + 1],
                )

            # Software-pipelined by one tile, with the DVE queue ordered
            # [reduce(t-1), L1b(t), L2(t)]: GpSimd's L1a(t) fold starts
            # when DMA(t) lands, which is when DVE starts the (port-
            # light, contention-immune) reduce of t-1 -- so POOL never
            # overlaps a packed TT, and DVE never stalls on the POOL
            # semaphore (L2(t) issues after both L1 halves).
            pdum = resp.tile([1, 4], mybir.dt.bfloat16, tag="pdum")
            T = len(plan)
            pending = None
            for t, (r0, w, pc) in enumerate(plan):
                b = C // w          # pieces per row
                f = w * WP          # bf16 elems per partition
                # The scalar engine (own SBUF port -- no DVE contention,
                # unlike POOL) sums the last w/8 channels of big tiles
                # raw via Copy+accum (739 ns/channel measured), shaving
                # the DVE critical path.
                ac = (10 if w == 64 else 5) if w >= 32 else 0
                xt = xp.tile([128, max_w * WP], mybir.dt.bfloat16, tag="xt")
                # The scalar ring is only coupled to DVE progress once
                # Squares enter its queue (first Square issues after
                # R(t0)), so the first few ramp tiles can use it as a
                # second ring to land earlier; everything after goes on
                # sync only.
                if t < 4:
                    eng = getattr(nc, ("sync", "scalar")[t % 2])
                else:
                    eng = getattr(nc, rings[t % len(rings)])
                src = x[r0 : r0 + w // 2, :].rearrange("a (b f) -> (a b) f", b=b)
                eng.dma_start(out=xt[:, :f], in_=src)

                x3 = xt[:, :f].rearrange("p (g v) -> p g v", v=WP)
                wd = w - ac         # channels on the DVE tree
                cs = accp.tile([128, max_w], mybir.dt.bfloat16, tag="cs")
                if ac:
                    trc = accp.tile([128, WP], mybir.dt.float32, tag="trc")
                    with nc.allow_low_precision("bf16 sums; fp32 internal accum"):
                        for j in range(ac):
                            nc.scalar.activation(
                                trc[:, :WP],
                                x3[:, wd + j, :],
                                mybir.ActivationFunctionType.Copy,
                                accum_out=cs[:, wd + j : wd + j + 1],
                            )
                ft = fp.tile([128, max_w * h1], mybir.dt.bfloat16, tag="ft")
                f3 = ft[:, : wd * h1].rearrange("p (g v) -> p g v", v=h1)
                # L1a: GpSimd folds the first pc channels.  The dummy
                # read of gt(t-1) gates the fold on L2(t-1) completion,
                # phase-locking POOL to DVE's (contention-immune) reduce
                # window -- an unconstrained POOL start drifts onto DVE's
                # packed TTs and halves their throughput (shared SBUF
                # port, measured 2-4x TT inflation).
                if pc:
                    if pending is not None:
                        gprev = pending[5]
                        nc.gpsimd.tensor_add(
                            pdum[0:1, 0:4], gprev[0:1, 0:4], gprev[0:1, 0:4]
                        )
                    nc.gpsimd.tensor_add(
                        f3[:, 0:pc, :], x3[:, 0:pc, 0:h1], x3[:, 0:pc, h1:WP]
                    )
                if pending is not None:
                    finish(*pending[:5])
                # L1b: DVE folds the rest (2x packed TT).
                if pc < wd:
                    nc.vector.tensor_add(
                        f3[:, pc:wd, :], x3[:, pc:wd, 0:h1], x3[:, pc:wd, h1:WP]
                    )
                # L2: h1 -> h2 (DVE, packed; waits on both L1 halves).
                gt = gp.tile([128, max_w * h2], mybir.dt.bfloat16, tag="gt")
                g3 = gt[:, : wd * h2].rearrange("p (g v) -> p g v", v=h2)
                nc.vector.tensor_add(g3, f3[:, :, 0:h2], f3[:, :, h2:h1])
                pending = (t, w, wd, g3, cs, gt)
                # Flush the early osb columns mid-stream so the final
                # output DMA only covers the last few tiles.  Issued on
                # the SCALAR ring: there it sits right after the Squares
                # it depends on; on the sync ring it would queue the
                # remaining input doorbells behind compute.
                if t == len(plan) - 3:
                    nsplit = len(plan) - 4
                    nc.scalar.dma_start(
                        out=out[0:128, 0:nsplit], in_=osb[:, 0:nsplit]
                    )
            finish(*pending[:5])
            nsplit = len(plan) - 4
            nc.scalar.dma_start(
                out=out[0:128, nsplit:], in_=osb[:, nsplit:T]
            )
    nc.finalize()
    return nc


def _get_nc():
    global _NC_CACHE
    if _NC_CACHE is None:
        _NC_CACHE = _build_nc()
    return _NC_CACHE


def _norms_from_partials(partials):
    """partials: [128, N_TILES] per-core -> per-row norms [228].

    Each tile's column is the full per-partition sum of squares: the
    DVE-reduced channels and the ACT-offloaded channels both land in
    the same cs tile before the single per-tile Square.
    """
    nsq = np.zeros(ROWS_PER_CORE, dtype=np.float64)
    for t, (r0, w, _) in enumerate(TILE_PLAN):
        b = C // w
        ps = partials[:, t].astype(np.float64).reshape(w // 2, b).sum(axis=1)
        nsq[r0 : r0 + w // 2] += ps
    return np.sqrt(nsq) / float(HW)


def _zero_mask():
    mask = np.ones((M, M), dtype=np.float64)
    for i in range(NUM_SAMPLES):
        r = (1 + i) * NUM_NODES
        for c in range(1, M):
            if c % NUM_NODES != 0 and (c - 1) // NUM_NODES != i:
                mask[r, c] = 0.0
    return mask


def _pattern_mixer_np(mat, sigma, lin_w, lin_b, mixed_mat):
    mat = np.asarray(mat, np.float64)            # [5, 7, 7]
    sigma = np.asarray(sigma, np.float64)        # [4, 5, 1]
    lin_w = np.asarray(lin_w, np.float64)        # [4, 5]
    lin_b = np.asarray(lin_b, np.float64)        # [4]
    mixed_mat = np.asarray(mixed_mat, np.float64)  # [4, 57, 57]

    T2 = 2 * NUM_FRAME - 1  # 15
    dist = np.abs(np.arange(T2, dtype=np.float64) - (NUM_FRAME - 1))
    te = (1.0 / (np.sqrt(2.0 * np.pi) * sigma)) * np.exp(
        -(dist**2) / (2.0 * sigma**2)
    )  # [4, 5, 15]
    ce = 1.0 / (1.0 + np.exp(-te))
    mixed = (
        np.einsum("hbt,bnm,hb->hntm", ce, mat, lin_w)
        + lin_b[:, None, None, None]
    )
    mixed = np.maximum(mixed, 0.0).reshape(NUM_MIXED, NUM_NODES, T2 * NUM_NODES)
    blocks = [
        mixed[
            :,
            :,
            NUM_NODES * (NUM_SAMPLES - 1 - i) : NUM_NODES * (2 * NUM_SAMPLES - 1 - i),
        ]
        for i in range(NUM_SAMPLES)
    ]
    add_block = np.concatenate(blocks, axis=1)  # [4, 56, 56]
    mm = mixed_mat.copy()
    mm[:, 1:, 1:] += add_block
    mm *= _zero_mask()[None]
    deg = np.maximum(mm.sum(axis=2), 1.0) ** -0.5  # [4, 57]
    return (deg[:, :, None] * mm * deg[:, None, :]).astype(np.float32)


def kernel(mat, x, sigma, lin_w, lin_b, mixed_mat, alpha):
    global LAST_RESULT
    xf = np.asarray(x, dtype=np.float32).reshape(ROWS_TOTAL, C, HW)
    xs = np.zeros((ROWS_TOTAL, C, WP), dtype=ml_dtypes.bfloat16)
    xs[:, :, :HW] = xf.astype(ml_dtypes.bfloat16)
    xs = xs.reshape(ROWS_TOTAL, CWP)
    in_maps = [
        {"x": xs[i * ROWS_PER_CORE : (i + 1) * ROWS_PER_CORE]} for i in range(N_CORES)
    ]
    nc = _get_nc()
    res = run_bass_kernel_spmd(nc, in_maps, core_ids=list(range(N_CORES)))
    LAST_RESULT = res
    norms = np.concatenate([_norms_from_partials(r["out"]) for r in res.results])
    nrm = norms.reshape(B, M).astype(np.float32)

    mp = _pattern_mixer_np(mat, sigma, lin_w, lin_b, mixed_mat)  # [4, 57, 57] f32
    alpha = np.asarray(alpha, np.float32).reshape(1, NUM_MIXED, 1, 1)
    out = mp[None] + alpha * nrm[:, None, None, :]
    return np.ascontiguousarray(out.astype(np.float32))
